# revision 9
# baseline (speedup 1.0000x reference)
"""Self-contained Trainium2 Bass kernel for MultiHeadAttention (B=2, L=2048,
H=1024, 16 heads x 64) returning (y, attn) like the reference.

Sharding: 8 cores = 2 batch groups x 4 head-groups (4 heads per core).
Per core: QKV projections (f32r matmuls), scores + softmax + attn output
write, attn@V via transposed scores, fc partial, ReduceScatter over the
4-core batch group, residual + LayerNorm on this core's 512 rows.
"""

import numpy as np

import concourse.bass as bass
import concourse.mybir as mybir
import concourse.tile as tile
from concourse import bacc
from concourse.bass_utils import run_bass_kernel_spmd
from concourse.masks import make_identity

F32 = mybir.dt.float32
F32R = mybir.dt.float32r
AF = mybir.ActivationFunctionType

N_HEADS = 16
HEAD_DIM = 64
HIDDEN = 1024
B = 2
L = 2048
LN_EPS = 1e-6

N_CORES = 8
GROUPS = 4            # cores per batch group (head groups)
HPC = N_HEADS // GROUPS  # heads per core = 4
PAIRS = HPC // 2      # head pairs per core = 2
FPC = HPC * HEAD_DIM  # features per core = 256

HC = HIDDEN // 128    # hidden chunks = 8
QT = L // 128         # q tiles = 16
KC = L // 512         # k chunks of 512 = 4
TCK = 256             # projection token chunk
NCK = L // TCK        # 8 token chunks
ROWS = L // GROUPS    # y rows per core = 512


def r32(ap):
    return ap


def build_program():
    nc = bacc.Bacc("TRN2", target_bir_lowering=False, debug=False,
                   num_devices=N_CORES)

    # ---- I/O (per core) ----
    qT = nc.dram_tensor("qT", [HIDDEN, L], F32R, kind="ExternalInput")
    kT = nc.dram_tensor("kT", [HIDDEN, L], F32R, kind="ExternalInput")
    vT = nc.dram_tensor("vT", [HIDDEN, L], F32R, kind="ExternalInput")
    # weights pre-rearranged on host into SBUF layouts
    wq = nc.dram_tensor("wq", [128, HC * FPC], F32R, kind="ExternalInput")
    wk = nc.dram_tensor("wk", [128, HC * FPC], F32R, kind="ExternalInput")
    wv = nc.dram_tensor("wv", [128, HC * FPC], F32R, kind="ExternalInput")
    wfc = nc.dram_tensor("wfc", [128, PAIRS * HIDDEN], F32R, kind="ExternalInput")
    resid = nc.dram_tensor("resid", [ROWS, HIDDEN], F32, kind="ExternalInput")
    gamma = nc.dram_tensor("gamma", [HIDDEN], F32, kind="ExternalInput")
    beta = nc.dram_tensor("beta", [HIDDEN], F32, kind="ExternalInput")

    attn_out = nc.dram_tensor("attn_out", [HPC, L, L], F32, kind="ExternalOutput")
    y_out = nc.dram_tensor("y_out", [ROWS, HIDDEN], F32, kind="ExternalOutput")

    with tile.TileContext(nc) as tc:
        build_body(nc, tc, qT, kT, vT, wq, wk, wv, wfc, resid, gamma, beta,
                   attn_out, y_out)
    nc.compile()
    return nc


def build_body(nc, tc, qT, kT, vT, wq, wk, wv, wfc, resid, gamma, beta,
               attn_out, y_out):
    ctx_pools = {}

    with (
        tc.tile_pool(name="weights", bufs=1) as wpool,
        tc.tile_pool(name="persist", bufs=1) as pers,
        tc.tile_pool(name="stream", bufs=2) as stream,
        tc.tile_pool(name="attnp", bufs=2) as attnp,
        tc.tile_pool(name="expp", bufs=4) as expp,
        tc.tile_pool(name="small", bufs=2) as small,
        tc.tile_pool(name="psum", bufs=4, space="PSUM") as psum,
        tc.tile_pool(name="psumo", bufs=1, space="PSUM") as psumo,
        tc.tile_pool(name="psumy", bufs=2, space="PSUM") as psumy,
        tc.tile_pool(name="dram", bufs=1, space="DRAM") as dram,
    ):
        # ---- persistent SBUF ----
        wq_sb = wpool.tile([128, HC * FPC], F32R, tag="wq")
        wk_sb = wpool.tile([128, HC * FPC], F32R, tag="wk")
        wv_sb = wpool.tile([128, HC * FPC], F32R, tag="wv")
        wfc_sb = wpool.tile([128, PAIRS * HIDDEN], F32R, tag="wfc")
        nc.sync.dma_start(wq_sb[:], wq[:])
        nc.sync.dma_start(wk_sb[:], wk[:])
        nc.sync.dma_start(wv_sb[:], wv[:])
        nc.sync.dma_start(wfc_sb[:], wfc[:])

        gb = wpool.tile([128, 2 * HIDDEN], F32, tag="gb")
        nc.sync.dma_start(gb[:, 0:HIDDEN], gamma.ap().unsqueeze(0).to_broadcast((128, HIDDEN)))
        nc.sync.dma_start(gb[:, HIDDEN:2 * HIDDEN],
                          beta.ap().unsqueeze(0).to_broadcast((128, HIDDEN)))

        ident = wpool.tile([128, 128], F32, tag="ident")
        make_identity(nc, ident[:])

        eps = wpool.tile([128, 1], F32, tag="eps")
        nc.vector.memset(eps[:], LN_EPS)

        # qhT/khT: [128 = 2 heads x 64 feat, L] per pair (feature-major)
        qhT = [pers.tile([128, L], F32R, tag=f"qhT{p}", name=f"qhT{p}") for p in range(PAIRS)]
        khT = [pers.tile([128, L], F32R, tag=f"khT{p}", name=f"khT{p}") for p in range(PAIRS)]
        # vh: [128 tok, 16 ktiles x 256 feat] (token-major, natural)
        vh = pers.tile([128, QT * FPC], F32R, tag="vh")
        # outT: [128 = 2 heads x 64 feat, L] per pair
        outT = [pers.tile([128, L], F32R, tag=f"outT{p}", name=f"outT{p}") for p in range(PAIRS)]
        # per-(pair,head,qt) row sums and reciprocals: col = pair*32+hh*16+qt
        rs_all = pers.tile([128, 2 * PAIRS * QT], F32, tag="rs_all")
        recip_all = pers.tile([128, 2 * PAIRS * QT], F32, tag="recip_all")
        bcast = [pers.tile([128, L], F32, tag=f"bcast{p}", name=f"bcast{p}") for p in range(PAIRS)]

        # ---- DRAM scratch ----
        fc_bounce = dram.tile([L, HIDDEN], F32)
        rs_out = dram.tile([ROWS, HIDDEN], F32)
        recipT_dram = dram.tile([2 * PAIRS * QT, 128], F32)

        # ================= Stage P: projections =================
        # qhT[pair][feat, tok] and khT: lhsT = weight chunk, rhs = xT chunk
        for name, x_dram, w_sb, dst, scale in (
            ("q", qT, wq_sb, qhT, 1.0 / np.sqrt(HEAD_DIM)),
            ("k", kT, wk_sb, khT, None),
        ):
            xv = x_dram.ap().rearrange("(c p) t -> p c t", p=128)
            for ck in range(NCK):
                xt = stream.tile([128, HC * TCK], F32R, tag="xt")
                xtv = xt[:].rearrange("p (c t) -> p c t", c=HC)
                nc.sync.dma_start(xtv, xv[:, :, ck * TCK:(ck + 1) * TCK])
                for p in range(PAIRS):
                    ps = psum.tile([128, 512], F32, tag="ps")
                    for c in range(HC):
                        nc.tensor.matmul(
                            ps[:, 0:TCK],
                            r32(w_sb[:, c * FPC + p * 128: c * FPC + (p + 1) * 128]),
                            r32(xtv[:, c, :]),
                            start=(c == 0), stop=(c == HC - 1),
                        )
                    dslice = dst[p][:, ck * TCK:(ck + 1) * TCK]
                    if scale is not None:
                        nc.vector.tensor_scalar_mul(dslice, ps[:, 0:TCK], scale)
                    else:
                        nc.vector.tensor_copy(dslice, ps[:, 0:TCK])

        # vh[tok, feat]: lhsT = vT tile (stationary), rhs = weight chunk
        xv = vT.ap().rearrange("(c p) t -> p c t", p=128)
        for ck in range(NCK):
            xt = stream.tile([128, HC * TCK], F32R, tag="xt")
            xtv = xt[:].rearrange("p (c t) -> p c t", c=HC)
            nc.sync.dma_start(xtv, xv[:, :, ck * TCK:(ck + 1) * TCK])
            for ts in range(TCK // 128):
                kt = ck * (TCK // 128) + ts
                ps = psum.tile([128, 512], F32, tag="ps")
                for c in range(HC):
                    nc.tensor.matmul(
                        ps[:, 0:FPC],
                        r32(xtv[:, c, ts * 128:(ts + 1) * 128]),
                        r32(wv_sb[:, c * FPC:(c + 1) * FPC]),
                        start=(c == 0), stop=(c == HC - 1),
                    )
                nc.vector.tensor_copy(vh[:, kt * FPC:(kt + 1) * FPC], ps[:, 0:FPC])

        # ================= Stage A: scores + softmax + attn out =================
        for p in range(PAIRS):
            for qt in range(QT):
                att = [attnp.tile([128, L], F32, tag="attn", name=f"att{p}_{qt}_{i}") for i in range(2)]
                rsp = [small.tile([128, 8], F32, tag="rsp", name=f"rsp{p}_{qt}_{i}") for i in range(2)]
                for kc in range(KC):
                    for hh in range(2):
                        ps = psum.tile([128, 512], F32, tag="ps")
                        nc.tensor.matmul(
                            ps[:],
                            r32(qhT[p][hh * 64:(hh + 1) * 64,
                                       qt * 128:(qt + 1) * 128]),
                            r32(khT[p][hh * 64:(hh + 1) * 64,
                                       kc * 512:(kc + 1) * 512]),
                            start=True, stop=True,
                        )
                        nc.scalar.activation(
                            att[hh][:, kc * 512:(kc + 1) * 512], ps[:],
                            AF.Exp, accum_out=rsp[hh][:, kc:kc + 1],
                        )
                for hh in range(2):
                    col = p * 32 + hh * 16 + qt
                    nc.vector.reduce_sum(rs_all[:, col:col + 1],
                                         rsp[hh][:, 0:KC],
                                         axis=mybir.AxisListType.X)
                    nc.vector.reciprocal(recip_all[:, col:col + 1],
                                         rs_all[:, col:col + 1])
                    nc.vector.tensor_scalar_mul(att[hh][:], att[hh][:],
                                                recip_all[:, col:col + 1])
                    nc.sync.dma_start(
                        attn_out[2 * p + hh, qt * 128:(qt + 1) * 128, :],
                        att[hh][:],
                    )

        # recipT: transpose recip_all -> [64, 128] rows=(pair,hh,qt), then
        # bounce via DRAM and broadcast each head's 16 rows across partitions.
        psT = psum.tile([128, 512], F32, tag="ps")
        nc.tensor.transpose(psT[0:2 * PAIRS * QT, 0:128], recip_all[:], ident[:])
        recipT_sb = small.tile([128, 128], F32, tag="recipT")
        nc.vector.tensor_copy(recipT_sb[0:2 * PAIRS * QT, :],
                              psT[0:2 * PAIRS * QT, 0:128])
        nc.sync.dma_start(recipT_dram[:], recipT_sb[0:2 * PAIRS * QT, :])
        for p in range(PAIRS):
            for hh in range(2):
                rows = recipT_dram[(p * 2 + hh) * QT:(p * 2 + hh + 1) * QT, :]
                nc.sync.dma_start(
                    bcast[p][hh * 64:(hh + 1) * 64, :],
                    rows.rearrange("a b -> (a b)").unsqueeze(0).to_broadcast((64, L)),
                )

        # ================= Stage A2: scoresT + attn@V =================
        for p in range(PAIRS):
            for qc in range(KC):
                po = [psumo.tile([64, 512], F32, tag=f"po{hh}", name=f"po{p}_{qc}_{hh}")
                      for hh in range(2)]
                for kt in range(QT):
                    ex = []
                    for hh in range(2):
                        ps = psum.tile([128, 512], F32, tag="ps")
                        nc.tensor.matmul(
                            ps[:],
                            r32(khT[p][hh * 64:(hh + 1) * 64,
                                       kt * 128:(kt + 1) * 128]),
                            r32(qhT[p][hh * 64:(hh + 1) * 64,
                                       qc * 512:(qc + 1) * 512]),
                            start=True, stop=True,
                        )
                        e = expp.tile([128, 512], F32R, tag="exp")
                        nc.scalar.activation(e[:], ps[:], AF.Exp)
                        ex.append(e)
                    for hh in range(2):
                        nc.tensor.matmul(
                            po[hh][:],
                            r32(vh[:, kt * FPC + p * 128 + hh * 64:
                                   kt * FPC + p * 128 + (hh + 1) * 64]),
                            r32(ex[hh][:]),
                            start=(kt == 0), stop=(kt == QT - 1),
                            skip_group_check=True,
                        )
                for hh in range(2):
                    nc.vector.tensor_mul(
                        outT[p][hh * 64:(hh + 1) * 64, qc * 512:(qc + 1) * 512],
                        po[hh][:],
                        bcast[p][hh * 64:(hh + 1) * 64, qc * 512:(qc + 1) * 512])

        # ================= Stage F: fc partial =================
        wfcv = wfc_sb[:].rearrange("p (a j) -> p a j", a=PAIRS)
        for qt in range(QT):
            ysb = small.tile([128, HIDDEN], F32, tag="ysb")
            for hc in range(2):
                py = psumy.tile([128, 512], F32, tag="py")
                for p in range(PAIRS):
                    nc.tensor.matmul(
                        py[:],
                        r32(outT[p][:, qt * 128:(qt + 1) * 128]),
                        r32(wfcv[:, p, hc * 512:(hc + 1) * 512]),
                        start=(p == 0), stop=(p == PAIRS - 1),
                    )
                nc.vector.tensor_copy(ysb[:, hc * 512:(hc + 1) * 512], py[:])
            nc.sync.dma_start(fc_bounce[qt * 128:(qt + 1) * 128, :], ysb[:])

        # ================= ReduceScatter =================
        nc.gpsimd.collective_compute(
            "ReduceScatter",
            mybir.AluOpType.add,
            ins=[fc_bounce.opt()],
            outs=[rs_out.opt()],
            replica_groups=[[0, 1, 2, 3], [4, 5, 6, 7]],
        )

        # ================= LayerNorm =================
        for qt in range(ROWS // 128):
            x = small.tile([128, HIDDEN], F32, tag="lnx")
            r = small.tile([128, HIDDEN], F32, tag="lnr")
            nc.sync.dma_start(x[:], rs_out[qt * 128:(qt + 1) * 128, :])
            nc.sync.dma_start(r[:], resid[qt * 128:(qt + 1) * 128, :])
            nc.vector.tensor_add(x[:], x[:], r[:])
            st = small.tile([128, 8], F32, tag="lnst")
            nc.vector.reduce_sum(st[:, 0:1], x[:], axis=mybir.AxisListType.X)
            nc.vector.tensor_scalar_mul(st[:, 1:2], st[:, 0:1], 1.0 / HIDDEN)
            c = small.tile([128, HIDDEN], F32, tag="lnc")
            nc.vector.tensor_scalar(c[:], x[:], st[:, 1:2], None,
                                    op0=mybir.AluOpType.subtract)
            # Square pass writes scratch into r (resid no longer needed)
            nc.scalar.activation(r[:], c[:], AF.Square,
                                 accum_out=st[:, 2:3])
            nc.vector.tensor_scalar_mul(st[:, 3:4], st[:, 2:3], 1.0 / HIDDEN)
            nc.scalar.activation(st[:, 4:5], st[:, 3:4], AF.Sqrt, bias=eps[:])
            nc.vector.reciprocal(st[:, 5:6], st[:, 4:5])
            nc.vector.tensor_scalar(x[:], c[:], st[:, 5:6], None,
                                    op0=mybir.AluOpType.mult)
            nc.vector.tensor_mul(x[:], x[:], gb[:, 0:HIDDEN])
            nc.vector.tensor_add(x[:], x[:], gb[:, HIDDEN:2 * HIDDEN])
            nc.sync.dma_start(y_out[qt * 128:(qt + 1) * 128, :], x[:])


_PROGRAM = None


def _get_program():
    global _PROGRAM
    if _PROGRAM is None:
        _PROGRAM = build_program()
    return _PROGRAM


def kernel(q, k, v, w_qs, w_ks, w_vs, w_fc, ln_gamma, ln_beta):
    q = np.asarray(q, np.float32)
    k = np.asarray(k, np.float32)
    v = np.asarray(v, np.float32)
    w_qs = np.asarray(w_qs, np.float32)
    w_ks = np.asarray(w_ks, np.float32)
    w_vs = np.asarray(w_vs, np.float32)
    w_fc = np.asarray(w_fc, np.float32)
    ln_gamma = np.asarray(ln_gamma, np.float32)
    ln_beta = np.asarray(ln_beta, np.float32)

    nc = _get_program()

    qTs = [np.ascontiguousarray(q[b].T) for b in range(B)]
    kTs = [np.ascontiguousarray(k[b].T) for b in range(B)]
    vTs = [np.ascontiguousarray(v[b].T) for b in range(B)]

    def w_proj_slices(w, hg):
        # [128, HC*FPC]: [p, c*FPC + f] = w[c*128+p, hg*FPC+f]
        sl = w[:, hg * FPC:(hg + 1) * FPC]           # [1024, 256]
        return np.ascontiguousarray(
            sl.reshape(HC, 128, FPC).transpose(1, 0, 2).reshape(128, HC * FPC))

    def w_fc_slices(w, hg):
        # [128, PAIRS*HIDDEN]: [p, pr*HIDDEN + j] = w[hg*FPC + pr*128 + p, j]
        sl = w[hg * FPC:(hg + 1) * FPC, :]           # [256, 1024]
        return np.ascontiguousarray(
            sl.reshape(PAIRS, 128, HIDDEN).transpose(1, 0, 2).reshape(128, -1))

    in_maps = []
    for c in range(N_CORES):
        b = c // GROUPS
        hg = c % GROUPS
        in_maps.append({
            "qT": qTs[b], "kT": kTs[b], "vT": vTs[b],
            "wq": w_proj_slices(w_qs, hg),
            "wk": w_proj_slices(w_ks, hg),
            "wv": w_proj_slices(w_vs, hg),
            "wfc": w_fc_slices(w_fc, hg),
            "resid": np.ascontiguousarray(q[b, hg * ROWS:(hg + 1) * ROWS, :]),
            "gamma": ln_gamma, "beta": ln_beta,
        })

    res = run_bass_kernel_spmd(nc, in_maps, list(range(N_CORES)))

    y = np.empty((B, L, HIDDEN), np.float32)
    attn = np.empty((B, N_HEADS, L, L), np.float32)
    for c in range(N_CORES):
        b = c // GROUPS
        hg = c % GROUPS
        out = res.results[c]
        y[b, hg * ROWS:(hg + 1) * ROWS, :] = out["y_out"]
        attn[b, hg * HPC:(hg + 1) * HPC] = out["attn_out"]
    return y, attn
